# revision 53
# baseline (speedup 1.0000x reference)
"""Trainium2 Bass kernel for nn_ASS_JRG_3573412790879 (gnn_message_passing).

Strategy: pure data parallelism over batch B=16 across 8 cores (2 samples/core).

Math reformulation (A=1 throughout):
  - The output only needs per-sample means over (T, 15 slots, J) of ReLU'd
    encoded features, then one small regression.
  - All graph contractions are linear and commute with the (linear part of
    the) encoders, so we encode feat_patch once per (W_comm0/W_diff0/W_diff1,
    idx) into 128-dim space and do the joint/time contraction there with
    host-precomputed expanded graph matrices (time-blocked, K<=128 via a
    lo/hi time split; the temporal shift of the "fp1" branch is folded into
    the expanded matrix).
  - relu+bias+sum-over-free-dim is a single fused op per PSUM bank
    (ACT activation(Relu, bias, accum_out) / DVE tensor_scalar(add,max,accum)).
  - fused = concat([s, s]) then @ W_reg.T  ==  s @ (W_reg[:, :128]+W_reg[:, 128:]).T;
    the 1/(15*T*J) mean scale is folded into that weight on the host.

Matmuls run in bf16 (PSUM accumulation is fp32); expected rel err ~1e-2 max.
"""
import numpy as np
import ml_dtypes
from contextlib import ExitStack

import concourse.bass as bass
import concourse.bacc as bacc
import concourse.tile as tile
from concourse import mybir
from concourse.bass_utils import run_bass_kernel_spmd

J, T, D, H, H1 = 17, 12, 400, 4, 128
LOT, HIT = 7, 6            # input t' counts for lo (t' 0..6) / hi (t' 6..11)
KLO, KHI = J * LOT, J * HIT  # 119, 102
NOUT = 6 * J               # output cols per half per map = 102
WC, W0, W1 = 5 * NOUT, 4 * NOUT, 4 * NOUT  # 510, 408, 408
GW_COLS = 1326             # WC+W0+W1
BF16 = mybir.dt.bfloat16
F32 = mybir.dt.float32
NPBF = ml_dtypes.bfloat16

GSLICE = {0: (0, WC), 1: (WC, W0), 2: (WC + W0, W1)}


# ---------------------------------------------------------------- host side
def _host_constants(inputs):
    jg = np.asarray(inputs['joint_graphs'], np.float64)
    sg = np.abs(np.asarray(inputs['gs_mats'], np.float64) * jg)
    tg = np.abs(np.asarray(inputs['gt_mats'], np.float64) * jg)
    mw_s = sg * np.asarray(inputs['s_jcw'], np.float64)[:, None, :, 0]
    mw_t = tg * np.asarray(inputs['t_jcw'], np.float64)[:, None, :, 0]
    rs_s, rs_t = mw_s.sum(2), mw_t.sum(2)

    def expand(t0, nin):
        # rows are t-major: r = (t'-t0)*J + j  (matches the t-major FP layout
        # so each half is ONE contiguous column span of the fpt tiles)
        K = J * nin
        Gc = np.zeros((K, WC)); G0 = np.zeros((K, W0)); G1 = np.zeros((K, W1))
        for t in range(6):
            gt = t0 + t
            ts = min(gt + 1, T - 1)
            for jj in range(J):
                r = (gt - t0) * J + jj
                rs = (ts - t0) * J + jj
                Gc[r, 0 * NOUT + jj * 6 + t] = 1.0           # ec0 identity map
                for h in range(H):
                    for ii in range(J):
                        c = ii * 6 + t
                        Gc[r, (1 + h) * NOUT + c] += sg[h, jj, ii]
                        G0[r, h * NOUT + c] += mw_s[h, ii, jj] - (rs_s[h, ii] if ii == jj else 0.0)
                        G1[rs, h * NOUT + c] += mw_t[h, ii, jj]
                        if ii == jj:
                            G1[r, h * NOUT + c] += -rs_t[h, ii]
        return np.concatenate([Gc, G0, G1], 1)

    gexp_lo = expand(0, LOT).astype(NPBF)   # [119, 1326]
    gexp_hi = expand(6, HIT).astype(NPBF)   # [102, 1326]

    Ws = [np.asarray(inputs['W_comm0']), np.asarray(inputs['W_diff0']),
          np.asarray(inputs['W_diff1'])]
    Wws = [np.asarray(inputs['W_whole']), np.asarray(inputs['W_diffw'])]
    # (idx, k)-major packing: one fused N=384 (enc) / N=256 (whole) matmul per
    # (idx, k) covers all types at once. wt = [100, 1536 enc | 1024 whole]
    wt = np.concatenate(
        [Ws[ty][idx][:, k * 100:(k + 1) * 100].T
         for idx in range(2) for k in range(2) for ty in range(3)]
        + [Wws[wt_][idx][:, k * 100:(k + 1) * 100].T
           for idx in range(2) for k in range(2) for wt_ in range(2)], 1).astype(NPBF)

    bs = [np.asarray(inputs['b_comm0']), np.asarray(inputs['b_diff0']),
          np.asarray(inputs['b_diff1'])]
    bw = [np.asarray(inputs['b_whole']), np.asarray(inputs['b_diffw'])]
    biasc = np.stack([bs[ty][idx] for ty in range(3) for idx in range(2)]
                     + [bw[wt][idx] for idx in range(2) for wt in range(2)],
                     1).astype(np.float32)   # [128, 10]; cols 6..9 = w0,dw0,w1,dw1

    Wr = np.asarray(inputs['W_reg'], np.float64)
    wrt = ((Wr[:, :H1] + Wr[:, H1:]) / (15.0 * T * J)).T.astype(NPBF)  # [128, 512]
    # combined graph tensor [119, 2652+12]: lo | hi (padded to 119 rows) | eye(12)
    gall = np.zeros((KLO, 2 * GW_COLS + T), NPBF)
    gall[:, :GW_COLS] = gexp_lo
    gall[:KHI, GW_COLS:2 * GW_COLS] = gexp_hi
    gall[:T, 2 * GW_COLS:] = np.eye(T).astype(NPBF)
    return dict(wt=wt, gall=gall, biasc=biasc,
                wrt=wrt, bregt=np.asarray(inputs['b_reg'])[None].astype(NPBF))


def _host_fp(inputs, b0):
    """[2, 400, 228] bf16, pre-transposed: cols 0:204 patch (t-major), 204:216
    FW^T, 216:228 FWshift^T."""
    fp = np.asarray(inputs['feat_patch'], np.float32)
    fw = np.asarray(inputs['feat_whole'], np.float32)
    out = np.empty((2, 400, 228), np.float32)
    for s in range(2):
        b = b0 + s
        out[s, :, :204] = fp[b].transpose(1, 0, 2).reshape(204, 400).T
        out[s, :, 204:216] = fw[b].T
        out[s, :, 216:227] = fw[b, 1:12].T
        out[s, :, 227] = fw[b, 11]
    # [100, 2, 4, 228]: per-partition-contiguous DMA layout; tile free dim is
    # (sample, dblock, col)
    return np.ascontiguousarray(
        out.reshape(2, 4, 100, 228).transpose(2, 0, 1, 3)).astype(NPBF)


# ---------------------------------------------------------------- device side
def _build_nc():
    nc = bacc.Bacc(None, target_bir_lowering=False, debug=False)
    d_fp = nc.dram_tensor("fp", [100, 2, 4, 228], BF16, kind="ExternalInput")
    d_wt = nc.dram_tensor("wt", [100, 2560], BF16, kind="ExternalInput")
    d_gall = nc.dram_tensor("gall", [KLO, 2 * GW_COLS + T], BF16, kind="ExternalInput")
    d_biasc = nc.dram_tensor("biasc", [128, 10], F32, kind="ExternalInput")
    d_wrt = nc.dram_tensor("wrt", [128, 512], BF16, kind="ExternalInput")
    d_bregt = nc.dram_tensor("bregt", [1, 512], BF16, kind="ExternalInput")
    d_out = nc.dram_tensor("out", [2, 512], F32, kind="ExternalOutput")

    with tile.TileContext(nc) as tc, ExitStack() as ctx:
        const = ctx.enter_context(tc.tile_pool(name="const", bufs=1))
        sb = ctx.enter_context(tc.tile_pool(name="sb", bufs=1))
        trash = ctx.enter_context(tc.tile_pool(name="trash", bufs=8))
        ps_e = ctx.enter_context(tc.tile_pool(name="ps_e", bufs=3, space="PSUM"))
        ps_g = ctx.enter_context(tc.tile_pool(name="ps_g", bufs=5, space="PSUM"))

        def cload(d, shape, dtype):
            t = const.tile(shape, dtype, tag=d.name)
            nc.sync.dma_start(t[:], d[:])
            return t

        # features first: sample-0 compute can start before the big graph
        # constants finish loading
        fpa = const.tile([100, 1824], BF16, tag="fpa")
        nc.sync.dma_start(fpa[:, 0:912], d_fp[:, 0].rearrange("p b c -> p (b c)"))
        t_wt = cload(d_wt, [100, 2560], BF16)
        nc.sync.dma_start(fpa[:, 912:1824], d_fp[:, 1].rearrange("p b c -> p (b c)"))
        t_gall = cload(d_gall, [KLO, 2 * GW_COLS + T], BF16)
        t_gexp = {'lo': t_gall[:, 0:GW_COLS], 'hi': t_gall[0:KHI, GW_COLS:2 * GW_COLS]}
        t_gw = t_gall[0:T, 2 * GW_COLS:2 * GW_COLS + T]
        t_biasc = cload(d_biasc, [128, 10], F32)
        t_wrt = cload(d_wrt, [128, 512], BF16)
        t_bregt = cload(d_bregt, [1, 512], BF16)

        t_m2 = const.tile([128, 2], BF16, tag="m2")
        t_ones2 = const.tile([1, 2], BF16, tag="ones2")
        nc.gpsimd.memset(t_ones2[:], 1.0)
        t_zeros = const.tile([128, 512], BF16, tag="zeros")
        nc.gpsimd.memset(t_zeros[:], 0.0)
        # prime the ACT spline-table load (~2.7us) so it overlaps the initial
        # DMAs instead of stalling the first real Relu
        nc.scalar.activation(t_zeros[0:1, 0:1], t_zeros[0:1, 0:1],
                             mybir.ActivationFunctionType.Relu)
        t_out = const.tile([2, 512], F32, tag="outT")

        for s in range(2):
            tg = lambda n: f"{n}_{s}"
            fpt = [fpa[:, (s * 4 + db) * 228:(s * 4 + db + 1) * 228] for db in range(4)]
            for db in range(4):
                ft = fpt[db]
                # FD^T = |FWs^T - FW^T| in cols 216:228 (overwrite FWs)
                nc.vector.tensor_tensor(ft[:, 216:228], ft[:, 216:228], ft[:, 204:216],
                                        mybir.AluOpType.subtract)
                nc.scalar.activation(ft[:, 216:228], ft[:, 216:228],
                                     mybir.ActivationFunctionType.Abs)

            # ---- whole-level encoder FIRST (fresh PSUM banks -> wait budget).
            # One fused N=256 matmul per (idx, k) covers both w/dw types.
            ew = ps_e.tile([12, 512], F32, tag="enc")
            for idx in range(2):
                for wt_ in range(2):
                    for k in range(2):
                        lhsT = fpt[idx * 2 + k][:, 204 + wt_ * 12:216 + wt_ * 12]
                        wc = 1536 + ((idx * 2 + k) * 2 + wt_) * 128
                        nc.tensor.matmul(ew[:, (idx * 2 + wt_) * 128:(idx * 2 + wt_ + 1) * 128],
                                         lhsT, t_wt[:, wc:wc + 128],
                                         start=(k == 0), stop=(k == 1))
            pw = sb.tile([12, 512], BF16, tag=tg("pw"))
            nc.vector.tensor_copy(pw[:], ew[:])

            # ---- patch encoders: P[(j,t'), c], one fused N=384 matmul per
            # (half, idx, k) covers all three types.
            P = {}
            for half, t0, nin, K in (('lo', 0, LOT, KLO), ('hi', 6, HIT, KHI)):
                bank_a = ps_e.tile([K, 384], F32, tag="enc")
                bank_b = ps_e.tile([K, 384], F32, tag="enc")
                banks = [bank_a, bank_b]
                c0_ = t0 * J
                for idx in range(2):
                    for k in range(2):
                        rhs = t_wt[:, (idx * 2 + k) * 384:(idx * 2 + k + 1) * 384]
                        nc.tensor.matmul(banks[idx][:], fpt[idx * 2 + k][:, c0_:c0_ + K],
                                         rhs, start=(k == 0), stop=(k == 1))
                p_a = sb.tile([K, 384], BF16, tag=tg(f"pa{half}"))
                nc.scalar.copy(p_a[:], bank_a[:])
                p_b = sb.tile([K, 384], BF16, tag=tg(f"pb{half}"))
                nc.vector.tensor_copy(p_b[:], bank_b[:])
                P[half] = [p_a, p_b]

            # ---- graph matmuls + fused relu/bias/accumulate
            part = sb.tile([128, 16], F32, tag=tg("part"))
            col = 0
            for half in ('lo', 'hi'):
                for ty in range(3):
                    for idx in range(2):
                        ptile = P[half][idx]
                        g0, gn = GSLICE[ty]
                        gps = ps_g.tile([128, 512], F32, tag="g")
                        nc.tensor.matmul(gps[:, 0:gn], ptile[:, ty * 128:(ty + 1) * 128],
                                         t_gexp[half][:, g0:g0 + gn], start=True, stop=True)
                        tr = trash.tile([128, 512], BF16, tag="trash")
                        bias_ap = t_biasc[:, ty * 2 + idx: ty * 2 + idx + 1]
                        if col % 2 == 0:
                            nc.scalar.activation(tr[:, 0:gn], gps[:, 0:gn],
                                                 mybir.ActivationFunctionType.Relu,
                                                 bias=bias_ap, accum_out=part[:, col:col + 1])
                        else:
                            nc.vector.scalar_tensor_tensor(
                                tr[:, 0:gn], gps[:, 0:gn], bias_ap, t_zeros[:, 0:gn],
                                mybir.AluOpType.add, mybir.AluOpType.max,
                                accum_out=part[:, col:col + 1])
                        col += 1
            # whole-level maps: out[c, t] blocks via eye(12), bias at relu
            gps = ps_g.tile([128, 512], F32, tag="g")
            for q in range(4):
                nc.tensor.matmul(gps[:, q * 12:(q + 1) * 12], pw[:, q * 128:(q + 1) * 128],
                                 t_gw[:], start=True, stop=True)
            tr = trash.tile([128, 512], BF16, tag="trash")
            for q in range(4):
                bias_ap = t_biasc[:, 6 + q:7 + q]
                if q % 2 == 0:
                    nc.scalar.activation(tr[:, q * 12:(q + 1) * 12], gps[:, q * 12:(q + 1) * 12],
                                         mybir.ActivationFunctionType.Relu,
                                         bias=bias_ap, accum_out=part[:, 12 + q:13 + q])
                else:
                    nc.vector.scalar_tensor_tensor(
                        tr[:, q * 12:(q + 1) * 12], gps[:, q * 12:(q + 1) * 12], bias_ap,
                        t_zeros[:, 0:12], mybir.AluOpType.add, mybir.AluOpType.max,
                        accum_out=part[:, 12 + q:13 + q])

            # ---- combine: M2[:, s] = sum(part[:,0:12]) + J*part[:,12]
            s1 = sb.tile([128, 1], F32, tag=tg("s1"))
            s2 = sb.tile([128, 1], F32, tag=tg("s2"))
            nc.vector.tensor_reduce(s1[:], part[:, 0:12], mybir.AxisListType.X,
                                    mybir.AluOpType.add)
            nc.vector.tensor_reduce(s2[:], part[:, 12:16], mybir.AxisListType.X,
                                    mybir.AluOpType.add)
            nc.vector.scalar_tensor_tensor(t_m2[:, s:s + 1], s2[:], float(J),
                                           s1[:], mybir.AluOpType.mult,
                                           mybir.AluOpType.add)

        # ---- regression for both samples: out = relu(M2.T @ wrt + breg)
        rg = ps_g.tile([2, 512], F32, tag="g")
        nc.tensor.matmul(rg[:], t_m2[:, 0:2], t_wrt[:], start=True, stop=False)
        nc.tensor.matmul(rg[:], t_ones2[:], t_bregt[:], start=False, stop=True)
        nc.scalar.activation(t_out[:], rg[:], mybir.ActivationFunctionType.Relu)
        nc.sync.dma_start(d_out[:], t_out[:])
    nc.compile()  # bacc legalization: split/move waits to satisfy HW sync budget
    return nc


_NC = None


def _get_nc():
    global _NC
    if _NC is None:
        _NC = _build_nc()
    return _NC


def _run(inputs, **kw):
    nc = _get_nc()
    C = _host_constants(inputs)
    in_maps = []
    for c in range(8):
        m = dict(C)
        m['fp'] = _host_fp(inputs, 2 * c)
        in_maps.append(m)
    res = run_bass_kernel_spmd(nc, in_maps, core_ids=list(range(8)), **kw)
    out = np.concatenate([r['out'] for r in res.results], 0)  # [16, 512]
    return out.reshape(16, 1, 512).astype(np.float32), res


def kernel(**inputs) -> np.ndarray:
    return _run(inputs)[0]


# revision 54
# speedup vs baseline: 1.0075x; 1.0075x over previous
"""Trainium2 Bass kernel for nn_ASS_JRG_3573412790879 (gnn_message_passing).

Strategy: pure data parallelism over batch B=16 across 8 cores (2 samples/core).

Math reformulation (A=1 throughout):
  - The output only needs per-sample means over (T, 15 slots, J) of ReLU'd
    encoded features, then one small regression.
  - All graph contractions are linear and commute with the (linear part of
    the) encoders, so we encode feat_patch once per (W_comm0/W_diff0/W_diff1,
    idx) into 128-dim space and do the joint/time contraction there with
    host-precomputed expanded graph matrices (time-blocked, K<=128 via a
    lo/hi time split; the temporal shift of the "fp1" branch is folded into
    the expanded matrix).
  - relu+bias+sum-over-free-dim is a single fused op per PSUM bank
    (ACT activation(Relu, bias, accum_out) / DVE tensor_scalar(add,max,accum)).
  - fused = concat([s, s]) then @ W_reg.T  ==  s @ (W_reg[:, :128]+W_reg[:, 128:]).T;
    the 1/(15*T*J) mean scale is folded into that weight on the host.

Matmuls run in bf16 (PSUM accumulation is fp32); expected rel err ~1e-2 max.
"""
import numpy as np
import ml_dtypes
from contextlib import ExitStack

import concourse.bass as bass
import concourse.bacc as bacc
import concourse.tile as tile
from concourse import mybir
from concourse.bass_utils import run_bass_kernel_spmd

J, T, D, H, H1 = 17, 12, 400, 4, 128
LOT, HIT = 7, 6            # input t' counts for lo (t' 0..6) / hi (t' 6..11)
KLO, KHI = J * LOT, J * HIT  # 119, 102
NOUT = 6 * J               # output cols per half per map = 102
WC, W0, W1 = 5 * NOUT, 4 * NOUT, 4 * NOUT  # 510, 408, 408
GW_COLS = 1326             # WC+W0+W1
BF16 = mybir.dt.bfloat16
F32 = mybir.dt.float32
NPBF = ml_dtypes.bfloat16

GSLICE = {0: (0, WC), 1: (WC, W0), 2: (WC + W0, W1)}


# ---------------------------------------------------------------- host side
def _host_constants(inputs):
    jg = np.asarray(inputs['joint_graphs'], np.float64)
    sg = np.abs(np.asarray(inputs['gs_mats'], np.float64) * jg)
    tg = np.abs(np.asarray(inputs['gt_mats'], np.float64) * jg)
    mw_s = sg * np.asarray(inputs['s_jcw'], np.float64)[:, None, :, 0]
    mw_t = tg * np.asarray(inputs['t_jcw'], np.float64)[:, None, :, 0]
    rs_s, rs_t = mw_s.sum(2), mw_t.sum(2)

    def expand(t0, nin):
        # rows are t-major: r = (t'-t0)*J + j  (matches the t-major FP layout
        # so each half is ONE contiguous column span of the fpt tiles)
        K = J * nin
        Gc = np.zeros((K, WC)); G0 = np.zeros((K, W0)); G1 = np.zeros((K, W1))
        for t in range(6):
            gt = t0 + t
            ts = min(gt + 1, T - 1)
            for jj in range(J):
                r = (gt - t0) * J + jj
                rs = (ts - t0) * J + jj
                Gc[r, 0 * NOUT + jj * 6 + t] = 1.0           # ec0 identity map
                for h in range(H):
                    for ii in range(J):
                        c = ii * 6 + t
                        Gc[r, (1 + h) * NOUT + c] += sg[h, jj, ii]
                        G0[r, h * NOUT + c] += mw_s[h, ii, jj] - (rs_s[h, ii] if ii == jj else 0.0)
                        G1[rs, h * NOUT + c] += mw_t[h, ii, jj]
                        if ii == jj:
                            G1[r, h * NOUT + c] += -rs_t[h, ii]
        return np.concatenate([Gc, G0, G1], 1)

    gexp_lo = expand(0, LOT).astype(NPBF)   # [119, 1326]
    gexp_hi = expand(6, HIT).astype(NPBF)   # [102, 1326]

    Ws = [np.asarray(inputs['W_comm0']), np.asarray(inputs['W_diff0']),
          np.asarray(inputs['W_diff1'])]
    Wws = [np.asarray(inputs['W_whole']), np.asarray(inputs['W_diffw'])]
    # (idx, k)-major packing: one fused N=384 (enc) / N=256 (whole) matmul per
    # (idx, k) covers all types at once. wt = [100, 1536 enc | 1024 whole]
    wt = np.concatenate(
        [Ws[ty][idx][:, k * 100:(k + 1) * 100].T
         for idx in range(2) for k in range(2) for ty in range(3)]
        + [Wws[wt_][idx][:, k * 100:(k + 1) * 100].T
           for idx in range(2) for k in range(2) for wt_ in range(2)], 1).astype(NPBF)

    bs = [np.asarray(inputs['b_comm0']), np.asarray(inputs['b_diff0']),
          np.asarray(inputs['b_diff1'])]
    bw = [np.asarray(inputs['b_whole']), np.asarray(inputs['b_diffw'])]
    biasc = np.stack([bs[ty][idx] for ty in range(3) for idx in range(2)]
                     + [bw[wt][idx] for idx in range(2) for wt in range(2)],
                     1).astype(np.float32)   # [128, 10]; cols 6..9 = w0,dw0,w1,dw1

    Wr = np.asarray(inputs['W_reg'], np.float64)
    wrt = ((Wr[:, :H1] + Wr[:, H1:]) / (15.0 * T * J)).T.astype(NPBF)  # [128, 512]
    # combined graph tensor [119, 2652+12]: lo | hi (padded to 119 rows) | eye(12)
    gall = np.zeros((KLO, 2 * GW_COLS + T), NPBF)
    gall[:, :GW_COLS] = gexp_lo
    gall[:KHI, GW_COLS:2 * GW_COLS] = gexp_hi
    gall[:T, 2 * GW_COLS:] = np.eye(T).astype(NPBF)
    return dict(wt=wt, gall=gall, biasc=biasc,
                wrt=wrt, bregt=np.asarray(inputs['b_reg'])[None].astype(NPBF))


def _host_fp(inputs, b0):
    """[2, 400, 228] bf16, pre-transposed: cols 0:204 patch (t-major), 204:216
    FW^T, 216:228 FWshift^T."""
    fp = np.asarray(inputs['feat_patch'], np.float32)
    fw = np.asarray(inputs['feat_whole'], np.float32)
    out = np.empty((2, 400, 228), np.float32)
    for s in range(2):
        b = b0 + s
        out[s, :, :204] = fp[b].transpose(1, 0, 2).reshape(204, 400).T
        out[s, :, 204:216] = fw[b].T
        out[s, :, 216:227] = fw[b, 1:12].T
        out[s, :, 227] = fw[b, 11]
    # [100, 2, 4, 228]: per-partition-contiguous DMA layout; tile free dim is
    # (sample, dblock, col)
    return np.ascontiguousarray(
        out.reshape(2, 4, 100, 228).transpose(2, 0, 1, 3)).astype(NPBF)


# ---------------------------------------------------------------- device side
def _build_nc():
    nc = bacc.Bacc(None, target_bir_lowering=False, debug=False)
    d_fp = nc.dram_tensor("fp", [100, 2, 4, 228], BF16, kind="ExternalInput")
    d_wt = nc.dram_tensor("wt", [100, 2560], BF16, kind="ExternalInput")
    d_gall = nc.dram_tensor("gall", [KLO, 2 * GW_COLS + T], BF16, kind="ExternalInput")
    d_biasc = nc.dram_tensor("biasc", [128, 10], F32, kind="ExternalInput")
    d_wrt = nc.dram_tensor("wrt", [128, 512], BF16, kind="ExternalInput")
    d_bregt = nc.dram_tensor("bregt", [1, 512], BF16, kind="ExternalInput")
    d_out = nc.dram_tensor("out", [2, 512], F32, kind="ExternalOutput")

    with tile.TileContext(nc) as tc, ExitStack() as ctx:
        const = ctx.enter_context(tc.tile_pool(name="const", bufs=1))
        sb = ctx.enter_context(tc.tile_pool(name="sb", bufs=1))
        trash = ctx.enter_context(tc.tile_pool(name="trash", bufs=8))
        ps_e = ctx.enter_context(tc.tile_pool(name="ps_e", bufs=3, space="PSUM"))
        ps_g = ctx.enter_context(tc.tile_pool(name="ps_g", bufs=5, space="PSUM"))

        def cload(d, shape, dtype):
            t = const.tile(shape, dtype, tag=d.name)
            nc.sync.dma_start(t[:], d[:])
            return t

        # features first: sample-0 compute can start before the big graph
        # constants finish loading
        fpa = const.tile([100, 1824], BF16, tag="fpa")
        nc.sync.dma_start(fpa[:, 0:912], d_fp[:, 0].rearrange("p b c -> p (b c)"))
        t_wt = cload(d_wt, [100, 2560], BF16)
        nc.sync.dma_start(fpa[:, 912:1824], d_fp[:, 1].rearrange("p b c -> p (b c)"))
        t_gall = cload(d_gall, [KLO, 2 * GW_COLS + T], BF16)
        t_gexp = {'lo': t_gall[:, 0:GW_COLS], 'hi': t_gall[0:KHI, GW_COLS:2 * GW_COLS]}
        t_gw = t_gall[0:T, 2 * GW_COLS:2 * GW_COLS + T]
        t_biasc = cload(d_biasc, [128, 10], F32)
        t_wrt = cload(d_wrt, [128, 512], BF16)
        t_bregt = cload(d_bregt, [1, 512], BF16)

        t_m2 = const.tile([128, 2], BF16, tag="m2")
        t_ones2 = const.tile([1, 2], BF16, tag="ones2")
        nc.gpsimd.memset(t_ones2[:], 1.0)
        t_zeros = const.tile([128, 512], BF16, tag="zeros")
        nc.gpsimd.memset(t_zeros[:], 0.0)
        # prime the ACT spline-table load (~2.7us) so it overlaps the initial
        # DMAs instead of stalling the first real Relu
        nc.scalar.activation(t_zeros[0:1, 0:1], t_zeros[0:1, 0:1],
                             mybir.ActivationFunctionType.Relu)
        t_out = const.tile([2, 512], F32, tag="outT")

        for s in range(2):
            tg = lambda n: f"{n}_{s}"
            fpt = [fpa[:, (s * 4 + db) * 228:(s * 4 + db + 1) * 228] for db in range(4)]
            for db in range(4):
                ft = fpt[db]
                # FD^T = |FWs^T - FW^T| in cols 216:228 (overwrite FWs)
                nc.vector.tensor_tensor(ft[:, 216:228], ft[:, 216:228], ft[:, 204:216],
                                        mybir.AluOpType.subtract)
                nc.vector.scalar_tensor_tensor(ft[:, 216:228], ft[:, 216:228], -1.0,
                                               ft[:, 216:228], mybir.AluOpType.mult,
                                               mybir.AluOpType.max)

            # ---- whole-level encoder FIRST (fresh PSUM banks -> wait budget).
            # One fused N=256 matmul per (idx, k) covers both w/dw types.
            ew = ps_e.tile([12, 512], F32, tag="enc")
            for idx in range(2):
                for wt_ in range(2):
                    for k in range(2):
                        lhsT = fpt[idx * 2 + k][:, 204 + wt_ * 12:216 + wt_ * 12]
                        wc = 1536 + ((idx * 2 + k) * 2 + wt_) * 128
                        nc.tensor.matmul(ew[:, (idx * 2 + wt_) * 128:(idx * 2 + wt_ + 1) * 128],
                                         lhsT, t_wt[:, wc:wc + 128],
                                         start=(k == 0), stop=(k == 1))
            pw = sb.tile([12, 512], BF16, tag=tg("pw"))
            nc.vector.tensor_copy(pw[:], ew[:])

            # ---- patch encoders: P[(j,t'), c], one fused N=384 matmul per
            # (half, idx, k) covers all three types.
            P = {}
            for half, t0, nin, K in (('lo', 0, LOT, KLO), ('hi', 6, HIT, KHI)):
                bank_a = ps_e.tile([K, 384], F32, tag="enc")
                bank_b = ps_e.tile([K, 384], F32, tag="enc")
                banks = [bank_a, bank_b]
                c0_ = t0 * J
                for idx in range(2):
                    for k in range(2):
                        rhs = t_wt[:, (idx * 2 + k) * 384:(idx * 2 + k + 1) * 384]
                        nc.tensor.matmul(banks[idx][:], fpt[idx * 2 + k][:, c0_:c0_ + K],
                                         rhs, start=(k == 0), stop=(k == 1))
                p_a = sb.tile([K, 384], BF16, tag=tg(f"pa{half}"))
                nc.scalar.copy(p_a[:], bank_a[:])
                p_b = sb.tile([K, 384], BF16, tag=tg(f"pb{half}"))
                nc.vector.tensor_copy(p_b[:], bank_b[:])
                P[half] = [p_a, p_b]

            # ---- graph matmuls + fused relu/bias/accumulate
            part = sb.tile([128, 16], F32, tag=tg("part"))
            col = 0
            for half in ('lo', 'hi'):
                for ty in range(3):
                    for idx in range(2):
                        ptile = P[half][idx]
                        g0, gn = GSLICE[ty]
                        gps = ps_g.tile([128, 512], F32, tag="g")
                        nc.tensor.matmul(gps[:, 0:gn], ptile[:, ty * 128:(ty + 1) * 128],
                                         t_gexp[half][:, g0:g0 + gn], start=True, stop=True)
                        tr = trash.tile([128, 512], BF16, tag="trash")
                        bias_ap = t_biasc[:, ty * 2 + idx: ty * 2 + idx + 1]
                        if col % 2 == 0:
                            nc.scalar.activation(tr[:, 0:gn], gps[:, 0:gn],
                                                 mybir.ActivationFunctionType.Relu,
                                                 bias=bias_ap, accum_out=part[:, col:col + 1])
                        else:
                            nc.vector.scalar_tensor_tensor(
                                tr[:, 0:gn], gps[:, 0:gn], bias_ap, t_zeros[:, 0:gn],
                                mybir.AluOpType.add, mybir.AluOpType.max,
                                accum_out=part[:, col:col + 1])
                        col += 1
            # whole-level maps: out[c, t] blocks via eye(12), bias at relu
            gps = ps_g.tile([128, 512], F32, tag="g")
            for q in range(4):
                nc.tensor.matmul(gps[:, q * 12:(q + 1) * 12], pw[:, q * 128:(q + 1) * 128],
                                 t_gw[:], start=True, stop=True)
            tr = trash.tile([128, 512], BF16, tag="trash")
            for q in range(4):
                bias_ap = t_biasc[:, 6 + q:7 + q]
                if q % 2 == 0:
                    nc.scalar.activation(tr[:, q * 12:(q + 1) * 12], gps[:, q * 12:(q + 1) * 12],
                                         mybir.ActivationFunctionType.Relu,
                                         bias=bias_ap, accum_out=part[:, 12 + q:13 + q])
                else:
                    nc.vector.scalar_tensor_tensor(
                        tr[:, q * 12:(q + 1) * 12], gps[:, q * 12:(q + 1) * 12], bias_ap,
                        t_zeros[:, 0:12], mybir.AluOpType.add, mybir.AluOpType.max,
                        accum_out=part[:, 12 + q:13 + q])

            # ---- combine: M2[:, s] = sum(part[:,0:12]) + J*part[:,12]
            s1 = sb.tile([128, 1], F32, tag=tg("s1"))
            s2 = sb.tile([128, 1], F32, tag=tg("s2"))
            nc.vector.tensor_reduce(s1[:], part[:, 0:12], mybir.AxisListType.X,
                                    mybir.AluOpType.add)
            nc.vector.tensor_reduce(s2[:], part[:, 12:16], mybir.AxisListType.X,
                                    mybir.AluOpType.add)
            nc.vector.scalar_tensor_tensor(t_m2[:, s:s + 1], s2[:], float(J),
                                           s1[:], mybir.AluOpType.mult,
                                           mybir.AluOpType.add)

        # ---- regression for both samples: out = relu(M2.T @ wrt + breg)
        rg = ps_g.tile([2, 512], F32, tag="g")
        nc.tensor.matmul(rg[:], t_m2[:, 0:2], t_wrt[:], start=True, stop=False)
        nc.tensor.matmul(rg[:], t_ones2[:], t_bregt[:], start=False, stop=True)
        nc.scalar.activation(t_out[:], rg[:], mybir.ActivationFunctionType.Relu)
        nc.sync.dma_start(d_out[:], t_out[:])
    nc.compile()  # bacc legalization: split/move waits to satisfy HW sync budget
    return nc


_NC = None


def _get_nc():
    global _NC
    if _NC is None:
        _NC = _build_nc()
    return _NC


def _run(inputs, **kw):
    nc = _get_nc()
    C = _host_constants(inputs)
    in_maps = []
    for c in range(8):
        m = dict(C)
        m['fp'] = _host_fp(inputs, 2 * c)
        in_maps.append(m)
    res = run_bass_kernel_spmd(nc, in_maps, core_ids=list(range(8)), **kw)
    out = np.concatenate([r['out'] for r in res.results], 0)  # [16, 512]
    return out.reshape(16, 1, 512).astype(np.float32), res


def kernel(**inputs) -> np.ndarray:
    return _run(inputs)[0]


# revision 55
# speedup vs baseline: 1.0517x; 1.0439x over previous
"""Trainium2 Bass kernel for nn_ASS_JRG_3573412790879 (gnn_message_passing).

Strategy: pure data parallelism over batch B=16 across 8 cores (2 samples/core).

Math reformulation (A=1 throughout):
  - The output only needs per-sample means over (T, 15 slots, J) of ReLU'd
    encoded features, then one small regression.
  - All graph contractions are linear and commute with the (linear part of
    the) encoders, so we encode feat_patch once per (W_comm0/W_diff0/W_diff1,
    idx) into 128-dim space and do the joint/time contraction there with
    host-precomputed expanded graph matrices (time-blocked, K<=128 via a
    lo/hi time split; the temporal shift of the "fp1" branch is folded into
    the expanded matrix).
  - relu+bias+sum-over-free-dim is a single fused op per PSUM bank
    (ACT activation(Relu, bias, accum_out) / DVE tensor_scalar(add,max,accum)).
  - fused = concat([s, s]) then @ W_reg.T  ==  s @ (W_reg[:, :128]+W_reg[:, 128:]).T;
    the 1/(15*T*J) mean scale is folded into that weight on the host.

Matmuls run in bf16 (PSUM accumulation is fp32); expected rel err ~1e-2 max.
"""
import numpy as np
import ml_dtypes
from contextlib import ExitStack

import concourse.bass as bass
import concourse.bacc as bacc
import concourse.tile as tile
from concourse import mybir
from concourse.bass_utils import run_bass_kernel_spmd

J, T, D, H, H1 = 17, 12, 400, 4, 128
LOT, HIT = 7, 6            # input t' counts for lo (t' 0..6) / hi (t' 6..11)
KLO, KHI = J * LOT, J * HIT  # 119, 102
NOUT = 6 * J               # output cols per half per map = 102
WC, W0, W1 = 5 * NOUT, 4 * NOUT, 4 * NOUT  # 510, 408, 408
GW_COLS = 1326             # WC+W0+W1
BF16 = mybir.dt.bfloat16
F32 = mybir.dt.float32
NPBF = ml_dtypes.bfloat16

GSLICE = {0: (0, WC), 1: (WC, W0), 2: (WC + W0, W1)}


# ---------------------------------------------------------------- host side
def _host_constants(inputs):
    jg = np.asarray(inputs['joint_graphs'], np.float64)
    sg = np.abs(np.asarray(inputs['gs_mats'], np.float64) * jg)
    tg = np.abs(np.asarray(inputs['gt_mats'], np.float64) * jg)
    mw_s = sg * np.asarray(inputs['s_jcw'], np.float64)[:, None, :, 0]
    mw_t = tg * np.asarray(inputs['t_jcw'], np.float64)[:, None, :, 0]
    rs_s, rs_t = mw_s.sum(2), mw_t.sum(2)

    def expand(t0, nin):
        # rows are t-major: r = (t'-t0)*J + j  (matches the t-major FP layout
        # so each half is ONE contiguous column span of the fpt tiles)
        K = J * nin
        Gc = np.zeros((K, WC)); G0 = np.zeros((K, W0)); G1 = np.zeros((K, W1))
        for t in range(6):
            gt = t0 + t
            ts = min(gt + 1, T - 1)
            for jj in range(J):
                r = (gt - t0) * J + jj
                rs = (ts - t0) * J + jj
                Gc[r, 0 * NOUT + jj * 6 + t] = 1.0           # ec0 identity map
                for h in range(H):
                    for ii in range(J):
                        c = ii * 6 + t
                        Gc[r, (1 + h) * NOUT + c] += sg[h, jj, ii]
                        G0[r, h * NOUT + c] += mw_s[h, ii, jj] - (rs_s[h, ii] if ii == jj else 0.0)
                        G1[rs, h * NOUT + c] += mw_t[h, ii, jj]
                        if ii == jj:
                            G1[r, h * NOUT + c] += -rs_t[h, ii]
        return np.concatenate([Gc, G0, G1], 1)

    gexp_lo = expand(0, LOT).astype(NPBF)   # [119, 1326]
    gexp_hi = expand(6, HIT).astype(NPBF)   # [102, 1326]

    Ws = [np.asarray(inputs['W_comm0']), np.asarray(inputs['W_diff0']),
          np.asarray(inputs['W_diff1'])]
    Wws = [np.asarray(inputs['W_whole']), np.asarray(inputs['W_diffw'])]
    # (idx, k)-major packing: one fused N=384 (enc) / N=256 (whole) matmul per
    # (idx, k) covers all types at once. wt = [100, 1536 enc | 1024 whole]
    wt = np.concatenate(
        [Ws[ty][idx][:, k * 100:(k + 1) * 100].T
         for idx in range(2) for k in range(2) for ty in range(3)]
        + [Wws[wt_][idx][:, k * 100:(k + 1) * 100].T
           for idx in range(2) for k in range(2) for wt_ in range(2)], 1).astype(NPBF)

    bs = [np.asarray(inputs['b_comm0']), np.asarray(inputs['b_diff0']),
          np.asarray(inputs['b_diff1'])]
    bw = [np.asarray(inputs['b_whole']), np.asarray(inputs['b_diffw'])]
    biasc = np.stack([bs[ty][idx] for ty in range(3) for idx in range(2)]
                     + [bw[wt][idx] for idx in range(2) for wt in range(2)],
                     1).astype(np.float32)   # [128, 10]; cols 6..9 = w0,dw0,w1,dw1

    Wr = np.asarray(inputs['W_reg'], np.float64)
    wrt = ((Wr[:, :H1] + Wr[:, H1:]) / (15.0 * T * J)).T.astype(NPBF)  # [128, 512]
    # combined graph tensor [119, 2652+12]: lo | hi (padded to 119 rows) | eye(12)
    gall = np.zeros((KLO, 2 * GW_COLS + T), NPBF)
    gall[:, :GW_COLS] = gexp_lo
    gall[:KHI, GW_COLS:2 * GW_COLS] = gexp_hi
    gall[:T, 2 * GW_COLS:] = np.eye(T).astype(NPBF)
    return dict(wt=wt, gall=gall, biasc=biasc,
                wrt=wrt, bregt=np.asarray(inputs['b_reg'])[None].astype(NPBF))


def _host_fp(inputs, b0):
    """[2, 400, 228] bf16, pre-transposed: cols 0:204 patch (t-major), 204:216
    FW^T, 216:228 FWshift^T."""
    fp = np.asarray(inputs['feat_patch'], np.float32)
    fw = np.asarray(inputs['feat_whole'], np.float32)
    out = np.empty((2, 400, 228), np.float32)
    for s in range(2):
        b = b0 + s
        out[s, :, :204] = fp[b].transpose(1, 0, 2).reshape(204, 400).T
        out[s, :, 204:216] = fw[b].T
        out[s, :, 216:227] = fw[b, 1:12].T
        out[s, :, 227] = fw[b, 11]
    # [100, 2, 4, 228]: per-partition-contiguous DMA layout; tile free dim is
    # (sample, dblock, col)
    return np.ascontiguousarray(
        out.reshape(2, 4, 100, 228).transpose(2, 0, 1, 3)).astype(NPBF)


# ---------------------------------------------------------------- device side
def _build_nc():
    nc = bacc.Bacc(None, target_bir_lowering=False, debug=False)
    d_fp = nc.dram_tensor("fp", [100, 2, 4, 228], BF16, kind="ExternalInput")
    d_wt = nc.dram_tensor("wt", [100, 2560], BF16, kind="ExternalInput")
    d_gall = nc.dram_tensor("gall", [KLO, 2 * GW_COLS + T], BF16, kind="ExternalInput")
    d_biasc = nc.dram_tensor("biasc", [128, 10], F32, kind="ExternalInput")
    d_wrt = nc.dram_tensor("wrt", [128, 512], BF16, kind="ExternalInput")
    d_bregt = nc.dram_tensor("bregt", [1, 512], BF16, kind="ExternalInput")
    d_out = nc.dram_tensor("out", [2, 512], F32, kind="ExternalOutput")

    with tile.TileContext(nc) as tc, ExitStack() as ctx:
        const = ctx.enter_context(tc.tile_pool(name="const", bufs=1))
        sb = ctx.enter_context(tc.tile_pool(name="sb", bufs=1))
        trash = ctx.enter_context(tc.tile_pool(name="trash", bufs=8))
        ps_e = ctx.enter_context(tc.tile_pool(name="ps_e", bufs=3, space="PSUM"))
        ps_g = ctx.enter_context(tc.tile_pool(name="ps_g", bufs=5, space="PSUM"))

        def cload(d, shape, dtype):
            t = const.tile(shape, dtype, tag=d.name)
            nc.sync.dma_start(t[:], d[:])
            return t

        # features first: sample-0 compute can start before the big graph
        # constants finish loading
        fpa = const.tile([100, 1824], BF16, tag="fpa")
        nc.sync.dma_start(fpa[:, 0:912], d_fp[:, 0].rearrange("p b c -> p (b c)"))
        t_wt = cload(d_wt, [100, 2560], BF16)
        nc.sync.dma_start(fpa[:, 912:1824], d_fp[:, 1].rearrange("p b c -> p (b c)"))
        t_gall = cload(d_gall, [KLO, 2 * GW_COLS + T], BF16)
        t_gexp = {'lo': t_gall[:, 0:GW_COLS], 'hi': t_gall[0:KHI, GW_COLS:2 * GW_COLS]}
        t_gw = t_gall[0:T, 2 * GW_COLS:2 * GW_COLS + T]
        t_biasc = cload(d_biasc, [128, 10], F32)
        t_wrt = cload(d_wrt, [128, 512], BF16)
        t_bregt = cload(d_bregt, [1, 512], BF16)

        t_m2 = const.tile([128, 2], BF16, tag="m2")
        t_ones2 = const.tile([1, 2], BF16, tag="ones2")
        nc.gpsimd.memset(t_ones2[:], 1.0)
        t_zeros = const.tile([128, 512], BF16, tag="zeros")
        nc.gpsimd.memset(t_zeros[:], 0.0)
        # prime the ACT spline-table load (~2.7us) so it overlaps the initial
        # DMAs instead of stalling the first real Relu
        nc.scalar.activation(t_zeros[0:1, 0:1], t_zeros[0:1, 0:1],
                             mybir.ActivationFunctionType.Relu)
        t_out = const.tile([2, 512], F32, tag="outT")

        for s in range(2):
            tg = lambda n: f"{n}_{s}"
            fpt = [fpa[:, (s * 4 + db) * 228:(s * 4 + db + 1) * 228] for db in range(4)]
            for db in range(4):
                ft = fpt[db]
                # FD^T = |FWs^T - FW^T| in cols 216:228 (overwrite FWs)
                nc.vector.tensor_tensor(ft[:, 216:228], ft[:, 216:228], ft[:, 204:216],
                                        mybir.AluOpType.subtract)
                nc.vector.scalar_tensor_tensor(ft[:, 216:228], ft[:, 216:228], -1.0,
                                               ft[:, 216:228], mybir.AluOpType.mult,
                                               mybir.AluOpType.max)

            # ---- whole-level encoder FIRST (fresh PSUM banks -> wait budget).
            # One fused N=256 matmul per (idx, k) covers both w/dw types.
            ew = ps_e.tile([12, 512], F32, tag="enc")
            for idx in range(2):
                for wt_ in range(2):
                    for k in range(2):
                        lhsT = fpt[idx * 2 + k][:, 204 + wt_ * 12:216 + wt_ * 12]
                        wc = 1536 + ((idx * 2 + k) * 2 + wt_) * 128
                        nc.tensor.matmul(ew[:, (idx * 2 + wt_) * 128:(idx * 2 + wt_ + 1) * 128],
                                         lhsT, t_wt[:, wc:wc + 128],
                                         start=(k == 0), stop=(k == 1))
            pw = sb.tile([12, 512], BF16, tag=tg("pw"))
            nc.vector.tensor_copy(pw[:], ew[:])

            # ---- patch encoders: P[(j,t'), c], one fused N=384 matmul per
            # (half, idx, k) covers all three types.
            P = {}
            for half, t0, nin, K in (('lo', 0, LOT, KLO), ('hi', 6, HIT, KHI)):
                bank_a = ps_e.tile([K, 384], F32, tag="enc")
                bank_b = ps_e.tile([K, 384], F32, tag="enc")
                banks = [bank_a, bank_b]
                c0_ = t0 * J
                for idx in range(2):
                    for k in range(2):
                        rhs = t_wt[:, (idx * 2 + k) * 384:(idx * 2 + k + 1) * 384]
                        nc.tensor.matmul(banks[idx][:], fpt[idx * 2 + k][:, c0_:c0_ + K],
                                         rhs, start=(k == 0), stop=(k == 1))
                p_a = sb.tile([K, 384], BF16, tag=tg(f"pa{half}"))
                nc.scalar.copy(p_a[:], bank_a[:])
                p_b = sb.tile([K, 384], BF16, tag=tg(f"pb{half}"))
                nc.vector.tensor_copy(p_b[:], bank_b[:])
                P[half] = [p_a, p_b]

            # ---- graph matmuls + fused relu/bias/accumulate
            part = sb.tile([128, 16], F32, tag=tg("part"))
            col = 0
            for half in ('lo', 'hi'):
                for ty in range(3):
                    for idx in range(2):
                        ptile = P[half][idx]
                        g0, gn = GSLICE[ty]
                        gps = ps_g.tile([128, 512], F32, tag="g")
                        nc.tensor.matmul(gps[:, 0:gn], ptile[:, ty * 128:(ty + 1) * 128],
                                         t_gexp[half][:, g0:g0 + gn], start=True, stop=True)
                        tr = trash.tile([128, 512], BF16, tag="trash")
                        bias_ap = t_biasc[:, ty * 2 + idx: ty * 2 + idx + 1]
                        if col % 2 == 0:
                            nc.scalar.activation(tr[:, 0:gn], gps[:, 0:gn],
                                                 mybir.ActivationFunctionType.Relu,
                                                 bias=bias_ap, accum_out=part[:, col:col + 1])
                        else:
                            nc.vector.scalar_tensor_tensor(
                                tr[:, 0:gn], gps[:, 0:gn], bias_ap, t_zeros[:, 0:gn],
                                mybir.AluOpType.add, mybir.AluOpType.max,
                                accum_out=part[:, col:col + 1])
                        col += 1
            # whole-level maps: out[c, t] blocks via eye(12), bias at relu
            gps = ps_g.tile([128, 512], F32, tag="g")
            for q in range(4):
                nc.tensor.matmul(gps[:, q * 12:(q + 1) * 12], pw[:, q * 128:(q + 1) * 128],
                                 t_gw[:], start=True, stop=True)
            tr = trash.tile([128, 512], BF16, tag="trash")
            for q in range(4):
                bias_ap = t_biasc[:, 6 + q:7 + q]
                nc.vector.scalar_tensor_tensor(
                    tr[:, q * 12:(q + 1) * 12], gps[:, q * 12:(q + 1) * 12], bias_ap,
                    t_zeros[:, 0:12], mybir.AluOpType.add, mybir.AluOpType.max,
                    accum_out=part[:, 12 + q:13 + q])

            # ---- combine: M2[:, s] = sum(part[:,0:12]) + J*part[:,12]
            s1 = sb.tile([128, 1], F32, tag=tg("s1"))
            s2 = sb.tile([128, 1], F32, tag=tg("s2"))
            nc.vector.tensor_reduce(s1[:], part[:, 0:12], mybir.AxisListType.X,
                                    mybir.AluOpType.add)
            nc.vector.tensor_reduce(s2[:], part[:, 12:16], mybir.AxisListType.X,
                                    mybir.AluOpType.add)
            nc.vector.scalar_tensor_tensor(t_m2[:, s:s + 1], s2[:], float(J),
                                           s1[:], mybir.AluOpType.mult,
                                           mybir.AluOpType.add)

        # ---- regression for both samples: out = relu(M2.T @ wrt + breg)
        rg = ps_g.tile([2, 512], F32, tag="g")
        nc.tensor.matmul(rg[:], t_m2[:, 0:2], t_wrt[:], start=True, stop=False)
        nc.tensor.matmul(rg[:], t_ones2[:], t_bregt[:], start=False, stop=True)
        nc.scalar.activation(t_out[:], rg[:], mybir.ActivationFunctionType.Relu)
        nc.sync.dma_start(d_out[:], t_out[:])
    nc.compile()  # bacc legalization: split/move waits to satisfy HW sync budget
    return nc


_NC = None


def _get_nc():
    global _NC
    if _NC is None:
        _NC = _build_nc()
    return _NC


def _run(inputs, **kw):
    nc = _get_nc()
    C = _host_constants(inputs)
    in_maps = []
    for c in range(8):
        m = dict(C)
        m['fp'] = _host_fp(inputs, 2 * c)
        in_maps.append(m)
    res = run_bass_kernel_spmd(nc, in_maps, core_ids=list(range(8)), **kw)
    out = np.concatenate([r['out'] for r in res.results], 0)  # [16, 512]
    return out.reshape(16, 1, 512).astype(np.float32), res


def kernel(**inputs) -> np.ndarray:
    return _run(inputs)[0]
